# revision 17
# baseline (speedup 1.0000x reference)
"""Trainium2 Bass kernel for 2x2 sliding-window entropy (nn_Entropy).

ent[c,h',w'] = -sum_i p_i*log(p_i+eps),  p_i = w_i/(S+eps),  S = sum_i w_i
over the 4 elements of each 2x2 window of x (stride 1).

Identity (exact up to the inner +eps):
    ent = u - B*R
    u = ln(S+eps), R = exp(-u) = 1/(S+eps), B = box2x2(G), G = x*ln(x+eps),
    S = box2x2(x)

Per core x is (64,256,256) -> flat rows (c*h)=16384 x 256; g-blocks of 128
input rows stepping 127 (1-row overlap) produce 127 output rows each;
16383/127 = 129 blocks. Super-blocks of GPER=16 g-blocks drive 1MB DMAs.

Both box dims on the PE: a [128,128] band matmul (bf16 -> FWL) does the
vertical pair-sum; a second matmul with the rhs shifted one column
accumulates into the same PSUM bank = full 2x2 box in PSUM (fp32).
PSUM groups of 4 g-blocks (4 banks, double-buffered): ACT drains u =
ln(S+eps) (FD 1024), DVE copies B out (2x copy mode) so the banks free
after ~1.3us and the PE never stalls long enough to lose the HAM clock.
R = exp(-u) on ACT; t2 = B*R on DVE (bf16 2x, super-block granularity);
ent = u - t2 on GpSimd. Everything off-PSUM lives in bf16: DMA traffic
halves and DVE runs its 2x packed mode; the rel-err cost (~1e-3) is well
inside the 2e-2 gate.

DMA notes (hardware-measured): per-partition contiguous chunks must be
>=2-8KB and transfers must span all 128 partitions, else SDMA throughput
collapses. The host therefore pre-transposes x into x_t[p,k,w] =
x[127k+p,w] (8KB bf16 chunks) and un-shuffles the raw block-dump output.

Sharding: pure data-parallel, batch dim (8) across the 8 cores.
"""
import numpy as np

B_FULL, C, H, W = 8, 64, 256, 256
HP, WP = H - 1, W - 1          # 255, 255
EPS = 1e-6
NCORES = 8

GROWS = 127                    # output rows per g-block
NG = (C * H - 1) // GROWS      # 16383/127 = 129 g-blocks
GPER = 16                      # max g-blocks per super-block
GROUP = 2                      # g-blocks per PSUM tile (1 pair = 2 banks)
# 8-wide super-blocks at both ends shorten pipeline fill/drain
SB_SIZES = [8, 8] + [16] * 6 + [8, 8, 1]

_CACHE = {}


def _build():
    import concourse.bacc as bacc
    import concourse.tile as tile
    import concourse.bass as bass
    import bass_rust as _bass_rust
    from concourse import mybir
    from concourse.hw_specs import get_activation_tables

    f32 = mybir.dt.float32
    bf16 = mybir.dt.bfloat16

    class _Bacc(bacc.Bacc):
        def insert_act_table_loads(self):
            # Ln and Exp both live in natural_log_exp_and_others; the default
            # greedy pick alternates two sets and reloads tables every block
            # (34 x 1.3us). Blank Ln/Exp from every other set (positions kept)
            # so both resolve to the combined set -> one load total.
            has_activation = any(
                isinstance(i, mybir.InstActivation)
                for b in self.main_func.blocks
                for i in b.instructions
            )
            if not has_activation:
                return
            LN = mybir.ActivationFunctionType.Ln
            EX = mybir.ActivationFunctionType.Exp
            items = []
            for name, fns in get_activation_tables(self.m.arch).items():
                if name != "natural_log_exp_and_others" and (LN in fns or EX in fns):
                    fns = fns - {LN, EX}
                items.append((name, fns))
            _bass_rust.insert_act_table_loads(self, items)

    nc = _Bacc("TRN2", target_bir_lowering=False, debug=False)

    x_d = nc.dram_tensor("x", [128 * NG * W], bf16, kind="ExternalInput")
    band_d = nc.dram_tensor("band", [128, 128], bf16, kind="ExternalInput")
    # W-wide rows (col 255 garbage, host slices); raw (sb, p, g) block order
    ent_d = nc.dram_tensor("ent", [(NG + 7) * 128 * W], bf16, kind="ExternalOutput")

    x_h = x_d[:].tensor
    ent_h = ent_d[:].tensor

    assert sum(SB_SIZES) == NG
    sblocks = []
    s = 0
    for sz in SB_SIZES:
        sblocks.append(list(range(s, s + sz)))
        s += sz
    NS = len(sblocks)
    offs = []
    off = 0
    for gs in sblocks:
        offs.append(off)
        off += 128 * len(gs) * W

    with tile.TileContext(nc) as tc:
        with (
            tc.tile_pool(name="singles", bufs=1) as singles,
            tc.tile_pool(name="comb", bufs=3) as comb_p,
            tc.tile_pool(name="lt", bufs=2) as lt_p,
            tc.tile_pool(name="ps", bufs=4, space="PSUM") as ps_p,
            tc.tile_pool(name="sb8", bufs=3) as sb8_p,
            tc.tile_pool(name="entp", bufs=2) as ent_p,
        ):
            band = singles.tile([128, 128], bf16)
            nc.gpsimd.dma_start(out=band, in_=band_d[:, :])
            eps_t = singles.tile([128, 1], f32)
            nc.vector.memset(eps_t, EPS)

            st = {}

            def stage_dma(idx):
                gs = sblocks[idx]
                gc = len(gs)
                n = gc * W
                xt = comb_p.tile([128, GPER * W + 4], bf16, tag="xt")
                # load x rows 127k..127k+127 for each k (host pre-transposed
                # to bf16: addr(p,k,w) = (p*NG + k)*W + w)
                src = bass.AP(
                    tensor=x_h,
                    offset=gs[0] * W,
                    ap=[[NG * W, 128], [1, n]],
                )
                nc.sync.dma_start(out=xt[:, 0:n], in_=src)
                st[idx] = {"xt": xt, "gc": gc, "n": n}

            def stage_l(idx):
                s = st[idx]
                n = s["n"]
                xt = s["xt"]
                L = lt_p.tile([128, GPER * W], bf16, tag="L")
                # L = ln(x + eps)   [ACT]
                nc.scalar.activation(
                    L[:, 0:n], xt[:, 0:n], mybir.ActivationFunctionType.Ln,
                    bias=eps_t,
                )
                s["L"] = L

            def stage_g(idx):
                s = st[idx]
                n = s["n"]
                Gt = comb_p.tile([128, GPER * W + 4], bf16, tag="Gt")
                # G = x * L   [DVE, bf16 2x]
                nc.vector.tensor_tensor(
                    Gt[:, 0:n], s["xt"][:, 0:n], s["L"][:, 0:n],
                    op=mybir.AluOpType.mult,
                )
                s["Gt"] = Gt

            def stage_groups(idx):
                s = st[idx]
                gc, n, xt, Gt = s["gc"], s["n"], s["xt"], s["Gt"]
                u8 = sb8_p.tile([128, GPER, W], bf16, tag="u8")
                B8 = sb8_p.tile([128, GPER, W], bf16, tag="B8")
                s["u8"], s["B8"] = u8, B8

                # PE: per group of 4 g-blocks, 4 PSUM banks hold the x and G
                # boxes of 2 pairs; vertical band matmul + column-shifted
                # accumulate = full 2x2 box in PSUM (fp32).
                for c0 in range(0, gc, GROUP):
                    cc = min(GROUP, gc - c0)
                    npair = (cc + 1) // 2
                    ps = ps_p.tile([128, 2 * npair, 2, W], f32, tag="ps")
                    for pp in range(npair):
                        g0 = c0 + 2 * pp
                        gn = min(2, gc - g0)
                        nn = gn * W
                        lo, hi = g0 * W, g0 * W + nn
                        px = ps[:, 2 * pp, 0:gn, :]
                        pg = ps[:, 2 * pp + 1, 0:gn, :]
                        nc.tensor.matmul(
                            px, band, xt[:, lo:hi], start=True, stop=False,
                        )
                        nc.tensor.matmul(
                            px, band, xt[:, lo + 1:hi + 1],
                            start=False, stop=False, skip_group_check=True,
                        )
                        nc.tensor.matmul(
                            pg, band, Gt[:, lo:hi],
                            start=True, stop=False, skip_group_check=True,
                        )
                        nc.tensor.matmul(
                            pg, band, Gt[:, lo + 1:hi + 1],
                            start=False, stop=(pp == npair - 1),
                            skip_group_check=True,
                        )
                    # dim1 of ps: even = x box (S), odd = G box (B)
                    gpp = cc // npair   # g-blocks per pair (2, or 1 for tail)
                    u_out = u8[:, c0:c0 + cc, :].rearrange(
                        "p (a b) w -> p a (b w)", a=npair
                    )
                    b_out = B8[:, c0:c0 + cc, :].rearrange(
                        "p (a b) w -> p a (b w)", a=npair
                    )
                    u_in = bass.AP(
                        tensor=ps.tensor, offset=ps.offset,
                        ap=[ps.ap[0], [4 * W, npair], [1, gpp * W]],
                    )
                    b_in = bass.AP(
                        tensor=ps.tensor, offset=ps.offset + 2 * W,
                        ap=[ps.ap[0], [4 * W, npair], [1, gpp * W]],
                    )
                    # u = ln(S+eps)  (PSUM -> SBUF)   [ACT]
                    nc.scalar.activation(
                        u_out, u_in, mybir.ActivationFunctionType.Ln,
                        bias=eps_t,
                    )
                    # B out of PSUM early (DVE cast) so the banks free
                    # without waiting for the u->R->t2 chain   [DVE]
                    nc.vector.tensor_copy(b_out, b_in)

            def stage_r(idx):
                s = st[idx]
                gc, u8 = s["gc"], s["u8"]
                R8 = sb8_p.tile([128, GPER, W], bf16, tag="R8")
                # R = exp(-u) = 1/(S+eps), whole super-block   [ACT]
                nc.scalar.activation(
                    R8[:, 0:gc, :], u8[:, 0:gc, :],
                    mybir.ActivationFunctionType.Exp, scale=-1.0,
                )
                s["R8"] = R8

            def stage_dve_tail(idx):
                s = st.pop(idx)
                gc, n, u8, B8, R8 = s["gc"], s["n"], s["u8"], s["B8"], s["R8"]
                t1 = ent_p.tile([128, GPER, W], bf16, tag="t1")
                ent8 = ent_p.tile([128, GPER, W], bf16, tag="ent8")
                # t2 = B * R  (drops the eps*u*R term, ~8e-5)   [DVE bf16 2x]
                nc.vector.tensor_tensor(
                    t1[:, 0:gc, :], R8[:, 0:gc, :], B8[:, 0:gc, :],
                    op=mybir.AluOpType.mult,
                )
                # ent = u - t2   [DVE bf16 2x; GpSimd would poison the shared
                # SBUF port and stretch concurrent DVE ops ~3x]
                nc.vector.tensor_tensor(
                    ent8[:, 0:gc, :], u8[:, 0:gc, :], t1[:, 0:gc, :],
                    op=mybir.AluOpType.subtract,
                )
                # raw contiguous dump: >=4KB contiguous per partition, full
                # 128 partitions; host un-shuffles
                dst = bass.AP(
                    tensor=ent_h,
                    offset=offs[idx],
                    ap=[[n, 128], [1, n]],
                )
                nc.sync.dma_start(
                    out=dst, in_=ent8[:, 0:gc, :].rearrange("p a b -> p (a b)")
                )

            # depth-4 software pipeline; every cross-engine input is >=1
            # iteration stale so no engine FIFO ever blocks on a same-
            # iteration producer. Per-iteration queue order:
            #   ACT: [L(i-1), u x8(i-2), R(i-2)]   ~11.8us
            #   DVE: [t2(i-3), ent(i-3), Bc x8(i-2), G(i-1)]   ~12.1us
            #   PE : MM groups(i-2); 2-bank PSUM tiles x4 bufs give PE a
            #        3-group runway over the u/Bc drains
            for idx in range(NS + 3):
                if idx < NS:
                    stage_dma(idx)
                if 1 <= idx <= NS:
                    stage_l(idx - 1)
                if idx >= 3:
                    stage_dve_tail(idx - 3)
                if 2 <= idx <= NS + 1:
                    stage_groups(idx - 2)
                if 1 <= idx <= NS:
                    stage_g(idx - 1)
                if 2 <= idx <= NS + 1:
                    stage_r(idx - 2)

    nc.compile()
    return nc


def _band_np():
    # [128,128] so bf16 weights hit the full-width fast-weight-load path;
    # out row 127 = in row 127 only (garbage, host drops it)
    a = np.zeros((128, 128), dtype=np.float32)
    for k in range(128):
        a[k, k] = 1.0
        if k > 0:
            a[k, k - 1] = 1.0
    return a


def _to_bf16(a):
    """fp32 -> bf16 round-to-nearest-even, returned as uint16 bit pattern."""
    b = np.ascontiguousarray(a, dtype=np.float32).view(np.uint32)
    r = (b + np.uint32(0x7FFF) + ((b >> np.uint32(16)) & np.uint32(1))) >> np.uint32(16)
    return r.astype(np.uint16)


def _make_in_maps(x: np.ndarray) -> list:
    """Per-core input maps: host pre-transpose + bf16 round of x."""
    from concourse import mybir
    bf = mybir.dt.np(mybir.dt.bfloat16)
    band = _to_bf16(_band_np()).view(bf)
    x = np.ascontiguousarray(x, dtype=np.float32)
    in_maps = []
    for i in range(NCORES):
        xf = x[i].reshape(C * H, W)
        rs = xf.strides[0]
        xt = np.lib.stride_tricks.as_strided(
            xf, shape=(128, NG, W), strides=(rs, GROWS * rs, xf.strides[1])
        )
        in_maps.append({
            "x": _to_bf16(xt).view(bf).reshape(-1),
            "band": band,
        })
    return in_maps


def kernel(x: np.ndarray) -> np.ndarray:
    from concourse.bass_utils import run_bass_kernel_spmd

    assert x.shape == (B_FULL, C, H, W), x.shape
    if "nc" not in _CACHE:
        _CACHE["nc"] = _build()
    nc = _CACHE["nc"]

    in_maps = _make_in_maps(x)
    res = run_bass_kernel_spmd(nc, in_maps, list(range(NCORES)))

    out = np.empty((NCORES, C * H, WP), dtype=np.float32)
    for i in range(NCORES):
        raw16 = np.asarray(res.results[i]["ent"]).view(np.uint16)
        raw = (raw16.astype(np.uint32) << np.uint32(16)).view(np.float32)
        parts = []
        off = 0
        for sz in SB_SIZES:
            blk = raw[off:off + 128 * sz * W].reshape(128, sz, W)
            parts.append(
                blk[:GROWS, :, :WP].transpose(1, 0, 2).reshape(sz * GROWS, WP)
            )
            off += 128 * sz * W
        out[i, : NG * GROWS] = np.concatenate(parts, axis=0)
    out = out.reshape(B_FULL, C, H, WP)[:, :, :HP, :]  # drop pad row 255
    return np.ascontiguousarray(out).reshape(B_FULL, C, HP * WP).astype(np.float32)


# revision 20
# speedup vs baseline: 1.2344x; 1.2344x over previous
"""Trainium2 Bass kernel for 2x2 sliding-window entropy (nn_Entropy).

ent[c,h',w'] = -sum_i p_i*log(p_i+eps),  p_i = w_i/(S+eps),  S = sum_i w_i
over the 4 elements of each 2x2 window of x (stride 1).

Identity (exact up to the inner +eps):
    ent = u - B*R
    u = ln(S+eps), R = exp(-u) = 1/(S+eps), B = box2x2(G), G = x*ln(x+eps),
    S = box2x2(x)

Per core x is (64,256,256) -> flat rows (c*h)=16384 x 256; g-blocks of 128
input rows stepping 127 (1-row overlap) produce 127 output rows each;
16383/127 = 129 blocks. Super-blocks of GPER=16 g-blocks drive 1MB DMAs.

Both box dims on the PE: a [128,128] band matmul (bf16 -> FWL) does the
vertical pair-sum; a second matmul with the rhs shifted one column
accumulates into the same PSUM bank = full 2x2 box in PSUM (fp32).
PSUM groups of 4 g-blocks (4 banks, double-buffered): ACT drains u =
ln(S+eps) (FD 1024), DVE copies B out (2x copy mode) so the banks free
after ~1.3us and the PE never stalls long enough to lose the HAM clock.
R = exp(-u) on ACT; t2 = B*R on DVE (bf16 2x, super-block granularity);
ent = u - t2 on GpSimd. Everything off-PSUM lives in bf16: DMA traffic
halves and DVE runs its 2x packed mode; the rel-err cost (~1e-3) is well
inside the 2e-2 gate.

DMA notes (hardware-measured): per-partition contiguous chunks must be
>=2-8KB and transfers must span all 128 partitions, else SDMA throughput
collapses. The host therefore pre-transposes x into x_t[p,k,w] =
x[127k+p,w] (8KB bf16 chunks) and un-shuffles the raw block-dump output.

Sharding: pure data-parallel, batch dim (8) across the 8 cores.
"""
import numpy as np

B_FULL, C, H, W = 8, 64, 256, 256
HP, WP = H - 1, W - 1          # 255, 255
EPS = 1e-6
NCORES = 8

GROWS = 127                    # output rows per g-block
NG = (C * H - 1) // GROWS      # 16383/127 = 129 g-blocks
GPER = 16                      # max g-blocks per super-block
GROUP = 4                      # g-blocks per PSUM tile pair (2 matmul pairs)
# 8-wide super-blocks at both ends shorten pipeline fill/drain
SB_SIZES = [8, 8] + [16] * 6 + [8, 8, 1]

_CACHE = {}


def _build():
    import concourse.bacc as bacc
    import concourse.tile as tile
    import concourse.bass as bass
    import bass_rust as _bass_rust
    from concourse import mybir
    from concourse.hw_specs import get_activation_tables

    f32 = mybir.dt.float32
    bf16 = mybir.dt.bfloat16

    class _Bacc(bacc.Bacc):
        def insert_act_table_loads(self):
            # Ln and Exp both live in natural_log_exp_and_others; the default
            # greedy pick alternates two sets and reloads tables every block
            # (34 x 1.3us). Blank Ln/Exp from every other set (positions kept)
            # so both resolve to the combined set -> one load total.
            has_activation = any(
                isinstance(i, mybir.InstActivation)
                for b in self.main_func.blocks
                for i in b.instructions
            )
            if not has_activation:
                return
            LN = mybir.ActivationFunctionType.Ln
            EX = mybir.ActivationFunctionType.Exp
            items = []
            for name, fns in get_activation_tables(self.m.arch).items():
                if name != "natural_log_exp_and_others" and (LN in fns or EX in fns):
                    fns = fns - {LN, EX}
                items.append((name, fns))
            _bass_rust.insert_act_table_loads(self, items)

    nc = _Bacc("TRN2", target_bir_lowering=False, debug=False)

    x_d = nc.dram_tensor("x", [128 * NG * W], bf16, kind="ExternalInput")
    band_d = nc.dram_tensor("band", [128, 128], bf16, kind="ExternalInput")
    # W-wide rows (col 255 garbage, host slices); raw (sb, p, g) block order
    ent_d = nc.dram_tensor("ent", [(NG + 7) * 128 * W], bf16, kind="ExternalOutput")

    x_h = x_d[:].tensor
    ent_h = ent_d[:].tensor

    assert sum(SB_SIZES) == NG
    sblocks = []
    s = 0
    for sz in SB_SIZES:
        sblocks.append(list(range(s, s + sz)))
        s += sz
    NS = len(sblocks)
    offs = []
    off = 0
    for gs in sblocks:
        offs.append(off)
        off += 128 * len(gs) * W

    with tile.TileContext(nc) as tc:
        with (
            tc.tile_pool(name="singles", bufs=1) as singles,
            tc.tile_pool(name="comb", bufs=3) as comb_p,
            tc.tile_pool(name="lt", bufs=2) as lt_p,
            tc.tile_pool(name="psx", bufs=2, space="PSUM") as psx_p,
            tc.tile_pool(name="psg", bufs=2, space="PSUM") as psg_p,
            tc.tile_pool(name="sb8", bufs=3) as sb8_p,
            tc.tile_pool(name="entp", bufs=2) as ent_p,
        ):
            band = singles.tile([128, 128], bf16)
            nc.gpsimd.dma_start(out=band, in_=band_d[:, :])
            eps_t = singles.tile([128, 1], f32)
            nc.vector.memset(eps_t, EPS)

            st = {}

            def stage_dma(idx):
                gs = sblocks[idx]
                gc = len(gs)
                n = gc * W
                xt = comb_p.tile([128, GPER * W + 4], bf16, tag="xt")
                # load x rows 127k..127k+127 for each k (host pre-transposed
                # to bf16: addr(p,k,w) = (p*NG + k)*W + w)
                src = bass.AP(
                    tensor=x_h,
                    offset=gs[0] * W,
                    ap=[[NG * W, 128], [1, n]],
                )
                nc.sync.dma_start(out=xt[:, 0:n], in_=src)
                st[idx] = {"xt": xt, "gc": gc, "n": n}

            def stage_l(idx):
                s = st[idx]
                n = s["n"]
                xt = s["xt"]
                L = lt_p.tile([128, GPER * W], bf16, tag="L")
                # L = ln(x + eps)   [ACT]
                nc.scalar.activation(
                    L[:, 0:n], xt[:, 0:n], mybir.ActivationFunctionType.Ln,
                    bias=eps_t,
                )
                s["L"] = L

            def stage_g(idx):
                s = st[idx]
                n = s["n"]
                Gt = comb_p.tile([128, GPER * W + 4], bf16, tag="Gt")
                # G = x * L   [DVE, bf16 2x]
                nc.vector.tensor_tensor(
                    Gt[:, 0:n], s["xt"][:, 0:n], s["L"][:, 0:n],
                    op=mybir.AluOpType.mult,
                )
                s["Gt"] = Gt

            def stage_groups(idx):
                s = st[idx]
                gc, n, xt, Gt = s["gc"], s["n"], s["xt"], s["Gt"]
                u8 = sb8_p.tile([128, GPER, W], bf16, tag="u8")
                B8 = sb8_p.tile([128, GPER, W], bf16, tag="B8")
                s["u8"], s["B8"] = u8, B8

                # PE: per group of 4 g-blocks, 4 PSUM banks hold the x and G
                # boxes of 2 pairs; vertical band matmul + column-shifted
                # accumulate = full 2x2 box in PSUM (fp32).
                for c0 in range(0, gc, GROUP):
                    cc = min(GROUP, gc - c0)
                    npair = (cc + 1) // 2
                    # x boxes and G boxes live in SEPARATE PSUM tiles so the
                    # two drains (ACT u on ps_x, DVE cast on ps_g) are not
                    # serialized by the same-tensor PSUM tracking.
                    ps_x = psx_p.tile([128, npair, 2, W], f32, tag="psx")
                    ps_g = psg_p.tile([128, npair, 2, W], f32, tag="psg")
                    for pp in range(npair):
                        g0 = c0 + 2 * pp
                        gn = min(2, gc - g0)
                        nn = gn * W
                        lo, hi = g0 * W, g0 * W + nn
                        px = ps_x[:, pp, 0:gn, :]
                        pg = ps_g[:, pp, 0:gn, :]
                        nc.tensor.matmul(
                            px, band, xt[:, lo:hi], start=True, stop=False,
                        )
                        nc.tensor.matmul(
                            px, band, xt[:, lo + 1:hi + 1],
                            start=False, stop=True, skip_group_check=True,
                        )
                        nc.tensor.matmul(
                            pg, band, Gt[:, lo:hi],
                            start=True, stop=False, skip_group_check=True,
                        )
                        nc.tensor.matmul(
                            pg, band, Gt[:, lo + 1:hi + 1],
                            start=False, stop=True, skip_group_check=True,
                        )
                    gpp = cc // npair   # g-blocks per pair (2, or 1 for tail)
                    u_out = u8[:, c0:c0 + cc, :].rearrange(
                        "p (a b) w -> p a (b w)", a=npair
                    )
                    b_out = B8[:, c0:c0 + cc, :].rearrange(
                        "p (a b) w -> p a (b w)", a=npair
                    )
                    # u = ln(S+eps)  (PSUM -> SBUF)   [ACT]
                    nc.scalar.activation(
                        u_out,
                        ps_x[:, 0:npair, 0:gpp, :].rearrange(
                            "p a b w -> p a (b w)"
                        ),
                        mybir.ActivationFunctionType.Ln,
                        bias=eps_t,
                    )
                    # B out of PSUM early (DVE cast) so the banks free
                    # without waiting for the u->R->t2 chain   [DVE]
                    nc.vector.tensor_copy(
                        b_out,
                        ps_g[:, 0:npair, 0:gpp, :].rearrange(
                            "p a b w -> p a (b w)"
                        ),
                    )

            def stage_r(idx):
                s = st[idx]
                gc, u8 = s["gc"], s["u8"]
                R8 = sb8_p.tile([128, GPER, W], bf16, tag="R8")
                # R = exp(-u) = 1/(S+eps), whole super-block   [ACT]
                nc.scalar.activation(
                    R8[:, 0:gc, :], u8[:, 0:gc, :],
                    mybir.ActivationFunctionType.Exp, scale=-1.0,
                )
                s["R8"] = R8

            def stage_dve_tail(idx):
                s = st.pop(idx)
                gc, n, u8, B8, R8 = s["gc"], s["n"], s["u8"], s["B8"], s["R8"]
                t1 = ent_p.tile([128, GPER, W], bf16, tag="t1")
                ent8 = ent_p.tile([128, GPER, W], bf16, tag="ent8")
                # t2 = B * R  (drops the eps*u*R term, ~8e-5)   [DVE bf16 2x]
                nc.vector.tensor_tensor(
                    t1[:, 0:gc, :], R8[:, 0:gc, :], B8[:, 0:gc, :],
                    op=mybir.AluOpType.mult,
                )
                # ent = u - t2   [DVE bf16 2x; GpSimd would poison the shared
                # SBUF port and stretch concurrent DVE ops ~3x]
                nc.vector.tensor_tensor(
                    ent8[:, 0:gc, :], u8[:, 0:gc, :], t1[:, 0:gc, :],
                    op=mybir.AluOpType.subtract,
                )
                # raw contiguous dump: >=4KB contiguous per partition, full
                # 128 partitions; host un-shuffles
                dst = bass.AP(
                    tensor=ent_h,
                    offset=offs[idx],
                    ap=[[n, 128], [1, n]],
                )
                nc.sync.dma_start(
                    out=dst, in_=ent8[:, 0:gc, :].rearrange("p a b -> p (a b)")
                )

            # depth-4 software pipeline; every cross-engine input is >=1
            # iteration stale so no engine FIFO ever blocks on a same-
            # iteration producer. Per-iteration queue order:
            #   ACT: [L(i-1), u x8(i-2), R(i-2)]   ~11.8us
            #   DVE: [t2(i-3), ent(i-3), Bc x8(i-2), G(i-1)]   ~12.1us
            #   PE : MM groups(i-2); 2-bank PSUM tiles x4 bufs give PE a
            #        3-group runway over the u/Bc drains
            for idx in range(NS + 3):
                if idx < NS:
                    stage_dma(idx)
                if 1 <= idx <= NS:
                    stage_l(idx - 1)
                if idx >= 3:
                    stage_dve_tail(idx - 3)
                if 2 <= idx <= NS + 1:
                    stage_groups(idx - 2)
                if 1 <= idx <= NS:
                    stage_g(idx - 1)
                if 2 <= idx <= NS + 1:
                    stage_r(idx - 2)

    nc.compile()
    return nc


def _band_np():
    # [128,128] so bf16 weights hit the full-width fast-weight-load path;
    # out row 127 = in row 127 only (garbage, host drops it)
    a = np.zeros((128, 128), dtype=np.float32)
    for k in range(128):
        a[k, k] = 1.0
        if k > 0:
            a[k, k - 1] = 1.0
    return a


def _to_bf16(a):
    """fp32 -> bf16 round-to-nearest-even, returned as uint16 bit pattern."""
    b = np.ascontiguousarray(a, dtype=np.float32).view(np.uint32)
    r = (b + np.uint32(0x7FFF) + ((b >> np.uint32(16)) & np.uint32(1))) >> np.uint32(16)
    return r.astype(np.uint16)


def _make_in_maps(x: np.ndarray) -> list:
    """Per-core input maps: host pre-transpose + bf16 round of x."""
    from concourse import mybir
    bf = mybir.dt.np(mybir.dt.bfloat16)
    band = _to_bf16(_band_np()).view(bf)
    x = np.ascontiguousarray(x, dtype=np.float32)
    in_maps = []
    for i in range(NCORES):
        xf = x[i].reshape(C * H, W)
        rs = xf.strides[0]
        xt = np.lib.stride_tricks.as_strided(
            xf, shape=(128, NG, W), strides=(rs, GROWS * rs, xf.strides[1])
        )
        in_maps.append({
            "x": _to_bf16(xt).view(bf).reshape(-1),
            "band": band,
        })
    return in_maps


def kernel(x: np.ndarray) -> np.ndarray:
    from concourse.bass_utils import run_bass_kernel_spmd

    assert x.shape == (B_FULL, C, H, W), x.shape
    if "nc" not in _CACHE:
        _CACHE["nc"] = _build()
    nc = _CACHE["nc"]

    in_maps = _make_in_maps(x)
    res = run_bass_kernel_spmd(nc, in_maps, list(range(NCORES)))

    out = np.empty((NCORES, C * H, WP), dtype=np.float32)
    for i in range(NCORES):
        raw16 = np.asarray(res.results[i]["ent"]).view(np.uint16)
        raw = (raw16.astype(np.uint32) << np.uint32(16)).view(np.float32)
        parts = []
        off = 0
        for sz in SB_SIZES:
            blk = raw[off:off + 128 * sz * W].reshape(128, sz, W)
            parts.append(
                blk[:GROWS, :, :WP].transpose(1, 0, 2).reshape(sz * GROWS, WP)
            )
            off += 128 * sz * W
        out[i, : NG * GROWS] = np.concatenate(parts, axis=0)
    out = out.reshape(B_FULL, C, H, WP)[:, :, :HP, :]  # drop pad row 255
    return np.ascontiguousarray(out).reshape(B_FULL, C, HP * WP).astype(np.float32)
